# revision 1
# baseline (speedup 1.0000x reference)
"""AttentionDecoder Trainium2 kernel — 8-core SPMD.

Strategy:
  - Data-parallel recurrence: core c owns batch slice [8c, 8c+8).
    LSTM+attention runs fully on-device; per-step gate matmuls stream the
    (replicated) recurrent weights as the moving operand in float32r.
  - One AllGather of the hidden states (bf16) after the recurrence.
  - Vocab-parallel output projection: core c computes preds[:, :, 4000c:4000c+4000]
    (bf16 matmul, fp32 accumulate), host concatenates the 8 vocab shards.

Host-side work is layout-only: shard/transpose/cast weights, fold LayerNorm
affine params and biases into adjacent matmuls. All math (LN, gather,
recurrence, attention, projection) runs on the NeuronCores.
"""

import os
import sys

sys.path.insert(0, "/opt/trn_rl_repo")

import ml_dtypes
import numpy as np

import concourse.bass as bass
from concourse import bacc
import concourse.mybir as mybir
import concourse.tile as tile
from concourse.bass_utils import run_bass_kernel_spmd
from concourse.masks import make_identity

# problem shapes (hardcoded per harness contract)
B, S, H, E, V, NL2, T = 64, 64, 512, 256, 32000, 4, 32
NCORES = 8
BL = B // NCORES  # 8 examples per core
VL = V // NCORES  # 4000 vocab rows per core
EPS = 1e-5
BS = BL * S  # 512 rows of encoder per core
TB = T * BL  # 256 (t, b) rows per core
G4 = 4 * H  # 2048 gate dim
BT = B * T  # 2048 gathered rows

F32 = mybir.dt.float32
F32R = mybir.dt.float32r
BF16 = mybir.dt.bfloat16
I32 = mybir.dt.int32
AF = mybir.ActivationFunctionType
ALU = mybir.AluOpType

bf16 = ml_dtypes.bfloat16


def _bc_free(ap, n):
    """Append a step-0 free dim of size n (broadcast along a new inner axis)."""
    return bass.AP(tensor=ap.tensor, offset=ap.offset, ap=[*ap.ap, [0, n]])


def _bc_col(ap, n):
    """[P, 1] column -> [P, n] broadcast (replace free dim with step-0)."""
    return bass.AP(tensor=ap.tensor, offset=ap.offset, ap=[ap.ap[0], [0, n]])


def build_nc(consts, debug=False):
    """Build the SPMD Bass program. consts: python-float immediates for the
    tiny init-projection weights."""
    nc = bacc.Bacc()

    # ---------------- DRAM I/O ----------------
    d_enc = nc.dram_tensor("enc", [BS, H], F32, kind="ExternalInput")
    d_ehnT = nc.dram_tensor("ehnT", [H, NL2 * BL], F32, kind="ExternalInput")
    d_ecn = nc.dram_tensor("ecn", [BL, NL2 * H], F32, kind="ExternalInput")
    d_emb = nc.dram_tensor("emb", [V, E], F32, kind="ExternalInput")
    d_tgt = nc.dram_tensor("tgt", [TB, 1], I32, kind="ExternalInput")
    d_kwT = nc.dram_tensor("kwT", [H, H], BF16, kind="ExternalInput")
    d_qwT = nc.dram_tensor("qwT", [H, H], BF16, kind="ExternalInput")
    d_ewT = nc.dram_tensor("ewT", [H, 1], BF16, kind="ExternalInput")
    d_qadd = nc.dram_tensor("qadd", [H, 1], F32, kind="ExternalInput")
    d_wzT = nc.dram_tensor("wzT", [2 * H, G4], F32R, kind="ExternalInput")
    d_xwT = nc.dram_tensor("xwT", [E + 1, G4], BF16, kind="ExternalInput")
    d_owT = nc.dram_tensor("owT", [H, VL], BF16, kind="ExternalInput")
    d_ob = nc.dram_tensor("ob", [1, VL], F32, kind="ExternalInput")
    d_out = nc.dram_tensor("out", [B, T, VL], F32, kind="ExternalOutput")

    # internal DRAM for the collective
    d_ccin = nc.dram_tensor("ccin", [H, TB], BF16)
    d_ccout = nc.dram_tensor("ccout", [NCORES * H, TB], BF16, addr_space="Shared")

    dbg = {}
    if debug:
        for nm, shp, dt_ in [
            ("dbg_encT", [H, BS], BF16), ("dbg_pkT", [H, BS], BF16),
            ("dbg_xg", [TB, G4], F32), ("dbg_h0T", [H, BL], F32),
            ("dbg_c0", [BL, H], F32), ("dbg_qT", [H, BL], BF16),
            ("dbg_tanh", [H, BS], BF16), ("dbg_alpha", [1, BS], F32),
            ("dbg_ctxT", [H, BL], F32), ("dbg_gates", [BL, G4], F32),
            ("dbg_hsT", [H, TB], F32),
            ("dbg_hsall", [H, BT], BF16),
            ("dbg_chunk", [128, 500], F32),
            ("dbg_xlnT", [E, TB], BF16),
        ]:
            dbg[nm] = nc.dram_tensor(nm, shp, dt_, kind="ExternalOutput")

    KT = H // 128  # 4 partition tiles for the 512 hidden dim

    with tile.TileContext(nc) as tc:
        with (
            tc.tile_pool(name="persist", bufs=1) as P_per,
            tc.tile_pool(name="recur", bufs=1) as P_rec,
            tc.tile_pool(name="cell", bufs=1) as P_cell,
            tc.tile_pool(name="psA", bufs=1, space="PSUM") as PS_a,
            tc.tile_pool(name="psG", bufs=2, space="PSUM") as PS_g,
        ):
            # ---------- persistent SBUF ----------
            id128 = P_per.tile([128, 128], F32, name="id128")
            make_identity(nc, id128[:, :])
            id8 = P_per.tile([8, 8], F32, name="id8")
            make_identity(nc, id8[:, :])
            eps_t = P_per.tile([128, 1], F32, name="eps")
            nc.vector.memset(eps_t[:, :], EPS)

            # hidden-state history (transposed): hs_T[kt] cols (t, b)
            hs_T = [P_per.tile([128, TB], F32R, name=f"hsT{k}") for k in range(KT)]
            hs_Tb = [P_per.tile([128, TB], BF16, name=f"hsTb{k}") for k in range(KT)]
            c_st = P_per.tile([BL, H], F32, name="c_state")

            # ---------- weights in SBUF (recurrence scope) ----------
            kwT = [P_rec.tile([128, H], BF16, name=f"kwT{k}") for k in range(KT)]
            qwT = [P_rec.tile([128, H], BF16, name=f"qwT{k}") for k in range(KT)]
            ewT = [P_rec.tile([128, 1], BF16, name=f"ewT{k}") for k in range(KT)]
            qadd = [P_rec.tile([128, 1], F32, name=f"qadd{k}") for k in range(KT)]
            wzT = [P_rec.tile([128, G4], F32R, name=f"wzT{k}") for k in range(2 * KT)]
            for k in range(KT):
                nc.sync.dma_start(kwT[k][:, :], d_kwT[128 * k : 128 * (k + 1), :])
                nc.sync.dma_start(qwT[k][:, :], d_qwT[128 * k : 128 * (k + 1), :])
                nc.sync.dma_start(ewT[k][:, :], d_ewT[128 * k : 128 * (k + 1), :])
                nc.sync.dma_start(qadd[k][:, :], d_qadd[128 * k : 128 * (k + 1), :])
            for k in range(2 * KT):
                nc.sync.dma_start(wzT[k][:, :], d_wzT[128 * k : 128 * (k + 1), :])

            enc_Tb = [P_rec.tile([128, BS], BF16, name=f"encT{k}") for k in range(KT)]
            pk_Tb = [P_rec.tile([128, BS], BF16, name=f"pkT{g}") for g in range(KT)]
            x_gates = [P_rec.tile([128, G4], F32, name=f"xg{m}") for m in range(2)]
            h0_T = [P_rec.tile([128, BL], F32R, name=f"h0T{k}") for k in range(KT)]
            h0_Tb = [P_rec.tile([128, BL], BF16, name=f"h0Tb{k}") for k in range(KT)]

            # ========== precompute (scoped pools; freed before recurrence) ==========
            with (
                tc.tile_pool(name="pre", bufs=1) as P_pre,
                tc.tile_pool(name="prew", bufs=2) as P_pw,
            ):
                xwT = [P_pre.tile([128, G4], BF16, name=f"xwT{k}") for k in range(2)]
                xwTb = P_pre.tile([1, G4], BF16, name="xwTbias")
                for k in range(2):
                    nc.sync.dma_start(xwT[k][:, :], d_xwT[128 * k : 128 * (k + 1), :])
                nc.sync.dma_start(xwTb[:, :], d_xwT[2 * 128 : 2 * 128 + 1, :])

                # ----- h0 / c0 -----
                ehnT = [
                    P_pre.tile([128, NL2 * BL], F32, name=f"ehnT{k}") for k in range(KT)
                ]
                for k in range(KT):
                    nc.sync.dma_start(ehnT[k][:, :], d_ehnT[128 * k : 128 * (k + 1), :])
                ecn = P_pre.tile([BL, NL2 * H], F32, name="ecn")
                nc.sync.dma_start(ecn[:, :], d_ecn[:, :])

                phw, pcw = consts["phw"], consts["pcw"]
                for k in range(KT):
                    tmp = P_pw.tile([128, BL], F32, name="h0tmp")
                    nc.vector.tensor_scalar_mul(
                        h0_T[k][:, :], ehnT[k][:, 0:BL], float(phw[0])
                    )
                    for l in range(1, NL2):
                        nc.vector.tensor_scalar_mul(
                            tmp[:, :], ehnT[k][:, BL * l : BL * (l + 1)], float(phw[l])
                        )
                        nc.vector.tensor_add(h0_T[k][:, :], h0_T[k][:, :], tmp[:, :])
                    nc.vector.tensor_scalar_add(
                        h0_T[k][:, :], h0_T[k][:, :], float(consts["phb"])
                    )
                    nc.vector.tensor_copy(h0_Tb[k][:, :], h0_T[k][:, :])

                ctmp = P_pw.tile([BL, H], F32, name="c0tmp")
                nc.vector.tensor_scalar_mul(c_st[:, :], ecn[:, 0:H], float(pcw[0]))
                for l in range(1, NL2):
                    nc.vector.tensor_scalar_mul(
                        ctmp[:, :], ecn[:, H * l : H * (l + 1)], float(pcw[l])
                    )
                    nc.vector.tensor_add(c_st[:, :], c_st[:, :], ctmp[:, :])
                nc.vector.tensor_scalar_add(
                    c_st[:, :], c_st[:, :], float(consts["pcb"])
                )

                # ----- encoder LN (natural layout) + transpose -----
                enc_ln = [P_pre.tile([128, H], F32, name=f"encln{i}") for i in range(4)]
                for i in range(BS // 128):
                    x_t = P_pw.tile([128, H], F32, name="enc_in")
                    nc.sync.dma_start(x_t[:, :], d_enc[128 * i : 128 * (i + 1), :])
                    stats = P_pw.tile([128, 6], F32, name="enc_st")
                    mv = P_pw.tile([128, 2], F32, name="enc_mv")
                    nc.vector.bn_stats(out=stats[:, :], in_=x_t[:, :])
                    nc.vector.bn_aggr(out=mv[:, :], in_=stats[:, :])
                    nc.scalar.activation(
                        out=mv[:, 1:2], in_=mv[:, 1:2], func=AF.Sqrt, bias=eps_t[:, :]
                    )
                    nc.vector.reciprocal(out=mv[:, 1:2], in_=mv[:, 1:2])
                    nc.vector.tensor_scalar(
                        out=enc_ln[i][:, :],
                        in0=x_t[:, :],
                        scalar1=mv[:, 0:1],
                        scalar2=mv[:, 1:2],
                        op0=ALU.subtract,
                        op1=ALU.mult,
                    )
                for i in range(4):
                    for j in range(4):
                        pt = PS_a.tile([128, 128], F32, name="tpose")
                        nc.tensor.transpose(
                            out=pt[:, :],
                            in_=enc_ln[i][:, 128 * j : 128 * (j + 1)],
                            identity=id128[:, :],
                        )
                        nc.vector.tensor_copy(
                            enc_Tb[j][:, 128 * i : 128 * (i + 1)], pt[:, :]
                        )

                # ----- projected keys pk_T (bf16) -----
                for g in range(KT):
                    pp = PS_g.tile([128, BS], F32, name="pk_ps", tag="mm_ps")
                    for k in range(KT):
                        nc.tensor.matmul(
                            pp[:, :],
                            lhsT=kwT[k][:, 128 * g : 128 * (g + 1)],
                            rhs=enc_Tb[k][:, :],
                            start=(k == 0),
                            stop=(k == KT - 1),
                        )
                    nc.vector.tensor_copy(pk_Tb[g][:, :], pp[:, :])

                # ----- embedding gather + LN + transpose -----
                xe_ln = [P_pre.tile([128, E], F32, name=f"xeln{i}") for i in range(2)]
                for i in range(2):
                    tgt_sb = P_pw.tile([128, 1], I32, name="tgt")
                    nc.sync.dma_start(tgt_sb[:, :], d_tgt[128 * i : 128 * (i + 1), :])
                    xg = P_pw.tile([128, E], F32, name="xemb")
                    nc.gpsimd.indirect_dma_start(
                        out=xg[:, :],
                        out_offset=None,
                        in_=d_emb[:, :],
                        in_offset=bass.IndirectOffsetOnAxis(ap=tgt_sb[:, 0:1], axis=0),
                    )
                    stats = P_pw.tile([128, 6], F32, name="xe_st")
                    mv = P_pw.tile([128, 2], F32, name="xe_mv")
                    nc.vector.bn_stats(out=stats[:, :], in_=xg[:, :])
                    nc.vector.bn_aggr(out=mv[:, :], in_=stats[:, :])
                    nc.scalar.activation(
                        out=mv[:, 1:2], in_=mv[:, 1:2], func=AF.Sqrt, bias=eps_t[:, :]
                    )
                    nc.vector.reciprocal(out=mv[:, 1:2], in_=mv[:, 1:2])
                    nc.vector.tensor_scalar(
                        out=xe_ln[i][:, :],
                        in0=xg[:, :],
                        scalar1=mv[:, 0:1],
                        scalar2=mv[:, 1:2],
                        op0=ALU.subtract,
                        op1=ALU.mult,
                    )
                xlnT = [P_pre.tile([128, TB], BF16, name=f"xlnT{k}") for k in range(2)]
                for i in range(2):
                    for j in range(2):
                        pt = PS_a.tile([128, 128], F32, name="tpose")
                        nc.tensor.transpose(
                            out=pt[:, :],
                            in_=xe_ln[i][:, 128 * j : 128 * (j + 1)],
                            identity=id128[:, :],
                        )
                        nc.vector.tensor_copy(
                            xlnT[j][:, 128 * i : 128 * (i + 1)], pt[:, :]
                        )
                ones_row = P_pre.tile([1, 128], BF16, name="ones")
                nc.vector.memset(ones_row[:, :], 1.0)
                if debug:
                    for k in range(2):
                        nc.sync.dma_start(dbg["dbg_xlnT"][128*k:128*(k+1), :], xlnT[k][:, :])

                # ----- x_gates = LN(emb[tgt]) @ w_x.T + b -----
                for m in range(2):
                    for b4 in range(4):
                        pp = PS_g.tile([128, 512], F32, name="xg_ps", tag="mm_ps")
                        for k in range(2):
                            nc.tensor.matmul(
                                pp[:, :],
                                lhsT=xlnT[k][:, 128 * m : 128 * (m + 1)],
                                rhs=xwT[k][:, 512 * b4 : 512 * (b4 + 1)],
                                start=(k == 0),
                                stop=False,
                            )
                        nc.tensor.matmul(
                            pp[:, :],
                            lhsT=ones_row[:, :],
                            rhs=xwTb[:, 512 * b4 : 512 * (b4 + 1)],
                            start=False,
                            stop=True,
                        )
                        nc.vector.tensor_copy(
                            x_gates[m][:, 512 * b4 : 512 * (b4 + 1)], pp[:, :]
                        )

            if debug:
                for k in range(KT):
                    nc.sync.dma_start(dbg["dbg_encT"][128*k:128*(k+1), :], enc_Tb[k][:, :])
                    nc.sync.dma_start(dbg["dbg_pkT"][128*k:128*(k+1), :], pk_Tb[k][:, :])
                    nc.sync.dma_start(dbg["dbg_h0T"][128*k:128*(k+1), :], h0_T[k][:, :].bitcast(F32))
                for m in range(2):
                    nc.sync.dma_start(dbg["dbg_xg"][128*m:128*(m+1), :], x_gates[m][:, :])
                nc.sync.dma_start(dbg["dbg_c0"][:, :], c_st[:, :])

            # ================= recurrence =================
            for t in range(T):
                if t == 0:
                    h_f32 = [h0_T[k][:, :] for k in range(KT)]
                    h_bf = [h0_Tb[k][:, :] for k in range(KT)]
                else:
                    h_f32 = [hs_T[k][:, BL * (t - 1) : BL * t] for k in range(KT)]
                    h_bf = [
                        hs_Tb[k][:, :].rearrange("p (b tt) -> p tt b", tt=T)[:, t - 1, :]
                        for k in range(KT)
                    ]

                # --- q_T = qw.T @ h (+qadd), bf16 ---
                q_Tb = [P_cell.tile([128, BL], BF16, name=f"qT{k}") for k in range(KT)]
                for g in range(KT):
                    pq = PS_a.tile([128, BL], F32, name="q_ps")
                    for k in range(KT):
                        nc.tensor.matmul(
                            pq[:, :],
                            lhsT=qwT[k][:, 128 * g : 128 * (g + 1)],
                            rhs=h_bf[k],
                            start=(k == 0),
                            stop=(k == KT - 1),
                        )
                    nc.vector.tensor_tensor(
                        out=q_Tb[g][:, :],
                        in0=pq[:, :],
                        in1=_bc_col(qadd[g][:, 0:1], BL),
                        op=ALU.add,
                    )

                # --- E = tanh(q + pk) (in-place tanh) ---
                esum = [P_cell.tile([128, BS], BF16, name=f"esum{g}") for g in range(KT)]
                for g in range(KT):
                    nc.vector.tensor_tensor(
                        out=esum[g][:, :].rearrange("p (b s) -> p b s", s=S),
                        in0=pk_Tb[g][:, :].rearrange("p (b s) -> p b s", s=S),
                        in1=_bc_free(q_Tb[g][:, :], S),
                        op=ALU.add,
                    )
                    nc.scalar.activation(
                        out=esum[g][:, :], in_=esum[g][:, :], func=AF.Tanh
                    )

                # --- energies = e_w . tanh -> [1, BS] psum ---
                pe = PS_a.tile([1, BS], F32, name="e_ps")
                for k in range(KT):
                    nc.tensor.matmul(
                        pe[:, :],
                        lhsT=ewT[k][:, :],
                        rhs=esum[k][:, :],
                        start=(k == 0),
                        stop=(k == KT - 1),
                    )

                # --- softmax over s (no max-subtraction; |e| bounded) ---
                expv = P_cell.tile([1, BS], F32, name="expv")
                nc.scalar.activation(out=expv[:, :], in_=pe[:, :], func=AF.Exp)
                ssum = P_cell.tile([1, BL], F32, name="ssum")
                nc.vector.tensor_reduce(
                    out=ssum[:, :],
                    in_=expv[:, :].rearrange("p (b s) -> p b s", s=S),
                    axis=mybir.AxisListType.X,
                    op=ALU.add,
                )
                nc.vector.reciprocal(out=ssum[:, :], in_=ssum[:, :])
                alpha = P_cell.tile([1, BS], BF16, name="alpha")
                nc.vector.tensor_tensor(
                    out=alpha[:, :].rearrange("p (b s) -> p b s", s=S),
                    in0=expv[:, :].rearrange("p (b s) -> p b s", s=S),
                    in1=_bc_free(ssum[:, :], S),
                    op=ALU.mult,
                )
                alpha_bc = P_cell.tile([128, BS], BF16, name="alpha_bc")
                nc.gpsimd.partition_broadcast(alpha_bc[:, :], alpha[:, :])
                if debug and t == 0:
                    for k in range(KT):
                        nc.sync.dma_start(dbg["dbg_qT"][128*k:128*(k+1), :], q_Tb[k][:, :])
                        nc.sync.dma_start(dbg["dbg_tanh"][128*k:128*(k+1), :], esum[k][:, :])
                    aex = P_cell.tile([1, BS], F32, name="aex")
                    nc.vector.tensor_copy(aex[:, :], alpha[:, :])
                    nc.sync.dma_start(dbg["dbg_alpha"][:, :], aex[:, :])

                # --- context_T[g, b] = sum_s alpha * enc_T ---
                ctx_T = [P_cell.tile([128, BL], F32R, name=f"ctxT{k}") for k in range(KT)]
                prod = P_cell.tile([128, BS], BF16, name="ctx_prod")
                for g in range(KT):
                    nc.vector.tensor_tensor(
                        out=prod[:, :],
                        in0=enc_Tb[g][:, :],
                        in1=alpha_bc[:, :],
                        op=ALU.mult,
                    )
                    with nc.allow_low_precision(reason="ctx f32r accum ok"):
                        nc.vector.tensor_reduce(
                            out=ctx_T[g][:, :],
                            in_=prod[:, :].rearrange("p (b s) -> p b s", s=S),
                            axis=mybir.AxisListType.X,
                            op=ALU.add,
                        )

                # --- gates = [ctx; h] @ w_z.T (f32r moving weights) + x_gates ---
                gates = P_cell.tile([BL, G4], F32, name="gates")
                xg_t = P_cell.tile([BL, G4], F32, name="xg_t")
                xrow = t % 16
                nc.sync.dma_start(
                    xg_t[:, :],
                    x_gates[t // 16][BL * xrow : BL * (xrow + 1), :],
                )
                for b4 in range(4):
                    pg = PS_g.tile([128, 512], F32, name="g_ps", tag="mm_ps")
                    for k in range(2 * KT):
                        lhs = ctx_T[k][:, :] if k < KT else h_f32[k - KT]
                        nc.tensor.matmul(
                            pg[0:BL, :],
                            lhsT=lhs,
                            rhs=wzT[k][:, 512 * b4 : 512 * (b4 + 1)],
                            start=(k == 0),
                            stop=(k == 2 * KT - 1),
                        )
                    nc.vector.tensor_tensor(
                        out=gates[:, 512 * b4 : 512 * (b4 + 1)],
                        in0=pg[0:BL, :],
                        in1=xg_t[:, 512 * b4 : 512 * (b4 + 1)],
                        op=ALU.add,
                    )

                if debug and t == 0:
                    for k in range(KT):
                        nc.sync.dma_start(dbg["dbg_ctxT"][128*k:128*(k+1), :], ctx_T[k][:, :].bitcast(F32))
                    nc.sync.dma_start(dbg["dbg_gates"][:, :], gates[:, :])
                # --- LSTM cell, in-place on gates slices ---
                g0, g1 = gates[:, 0:H], gates[:, H : 2 * H]
                g2, g3 = gates[:, 2 * H : 3 * H], gates[:, 3 * H : 4 * H]
                nc.scalar.activation(out=g0, in_=g0, func=AF.Sigmoid)
                nc.scalar.activation(out=g1, in_=g1, func=AF.Sigmoid)
                nc.scalar.activation(out=g2, in_=g2, func=AF.Tanh)
                nc.scalar.activation(out=g3, in_=g3, func=AF.Sigmoid)
                nc.vector.tensor_mul(g1, g1, c_st[:, :])  # sf*c
                nc.vector.tensor_mul(g0, g0, g2)  # si*tg
                nc.vector.tensor_add(c_st[:, :], g0, g1)  # c2
                nc.scalar.activation(out=g2, in_=c_st[:, :], func=AF.Tanh)
                h2 = P_cell.tile([BL, H], F32, name="h2")
                nc.vector.tensor_mul(h2[:, :], g3, g2)

                # --- transpose h2 -> hs_T / hs_Tb col t ---
                for k in range(KT):
                    pt = PS_a.tile([128, BL], F32, name="h_tpose")
                    nc.tensor.transpose(
                        out=pt[:, :],
                        in_=h2[:, 128 * k : 128 * (k + 1)],
                        identity=id8[:, :],
                    )
                    nc.vector.tensor_copy(hs_T[k][:, BL * t : BL * (t + 1)], pt[:, :])
                    hsb_v = hs_Tb[k][:, :].rearrange("p (b tt) -> p tt b", tt=T)
                    nc.vector.tensor_copy(hsb_v[:, t, :], pt[:, :])

            if debug:
                for k in range(KT):
                    nc.sync.dma_start(dbg["dbg_hsT"][128*k:128*(k+1), :], hs_T[k][:, :].bitcast(F32))
            # ================= AllGather hidden states =================
            for k in range(KT):
                nc.sync.dma_start(d_ccin[128 * k : 128 * (k + 1), :], hs_Tb[k][:, :])
            nc.gpsimd.collective_compute(
                "AllGather",
                ALU.bypass,
                replica_groups=[list(range(NCORES))],
                ins=[d_ccin[:, :]],
                outs=[d_ccout[:, :]],
            )

        # ================= projection phase =================
        with (
            tc.tile_pool(name="proj", bufs=1) as P_pj,
            tc.tile_pool(name="projw", bufs=3) as P_po,
            tc.tile_pool(name="psP", bufs=8, space="PSUM") as PS_p,
        ):
            hs_all = [P_pj.tile([128, BT], BF16, name=f"hsall{k}") for k in range(KT)]
            for k in range(KT):
                for r in range(NCORES):
                    nc.sync.dma_start(
                        hs_all[k][:, TB * r : TB * (r + 1)],
                        d_ccout[H * r + 128 * k : H * r + 128 * (k + 1), :],
                    )
            owT = [P_pj.tile([128, VL], BF16, name=f"owT{k}") for k in range(KT)]
            for k in range(KT):
                nc.sync.dma_start(owT[k][:, :], d_owT[128 * k : 128 * (k + 1), :])
            if debug:
                for k in range(KT):
                    nc.sync.dma_start(dbg["dbg_hsall"][128*k:128*(k+1), :], hs_all[k][:, :])
            ob_bc = P_pj.tile([128, VL], F32, name="ob_bc")
            ob_row = P_pj.tile([1, VL], F32, name="ob_row")
            nc.sync.dma_start(ob_row[:, :], d_ob[:, :])
            nc.gpsimd.partition_broadcast(ob_bc[:, :], ob_row[:, :])

            NV = 8  # vocab chunks
            VC = VL // NV  # 500
            for mt in range(BT // 128):
                for vc in range(NV):
                    pp = PS_p.tile([128, VC], F32, name="proj_ps")
                    for k in range(KT):
                        nc.tensor.matmul(
                            pp[:, :],
                            lhsT=hs_all[k][:, 128 * mt : 128 * (mt + 1)],
                            rhs=owT[k][:, VC * vc : VC * (vc + 1)],
                            start=(k == 0),
                            stop=(k == KT - 1),
                        )
                    ob_t = P_po.tile([128, VC], F32, name="proj_out")
                    nc.vector.tensor_tensor(
                        out=ob_t[:, :],
                        in0=pp[:, :],
                        in1=ob_bc[:, VC * vc : VC * (vc + 1)],
                        op=ALU.add,
                    )
                    if debug and mt == 0 and vc == 0:
                        nc.sync.dma_start(dbg["dbg_chunk"][:, :], ob_t[:, :])
                    # hs_all cols are (r, bl, t) => rows of out[(b t), v] are contiguous
                    dst = d_out[:, :, :].rearrange("b t v -> (b t) v")[
                        128 * mt : 128 * (mt + 1), VC * vc : VC * (vc + 1)
                    ]
                    nc.sync.dma_start(dst, ob_t[:, :])

    nc.compile()
    return nc


def _prep_inputs(inputs):
    """Host-side layout prep. Returns (in_maps, consts)."""
    f = lambda x: np.asarray(x, dtype=np.float32)
    targets = np.asarray(inputs["targets"])
    enc_hid = f(inputs["encoder_hidden"])
    enc_hn = f(inputs["enc_hn"])
    enc_cn = f(inputs["enc_cn"])
    emb = f(inputs["emb"])
    ln_enc_g = f(inputs["ln_enc_g"])
    ln_enc_b = f(inputs["ln_enc_b"])
    ln_emb_g = f(inputs["ln_emb_g"])
    ln_emb_b = f(inputs["ln_emb_b"])
    q_w = f(inputs["q_w"])
    q_b = f(inputs["q_b"])
    k_w = f(inputs["k_w"])
    e_w = f(inputs["e_w"])
    w_ih = f(inputs["w_ih"])
    w_hh = f(inputs["w_hh"])
    b_ih = f(inputs["b_ih"])
    b_hh = f(inputs["b_hh"])
    out_w = f(inputs["out_w"])
    out_b = f(inputs["out_b"])

    consts = dict(
        phw=[float(x) for x in f(inputs["proj_hn_w"])[0]],
        phb=float(f(inputs["proj_hn_b"])[0]),
        pcw=[float(x) for x in f(inputs["proj_cn_w"])[0]],
        pcb=float(f(inputs["proj_cn_b"])[0]),
    )

    # fold LN affines into adjacent matmuls
    kw_eff = k_w * ln_enc_g[None, :]
    qadd = q_b + k_w @ ln_enc_b
    w_ctx = w_ih[:, :H] * ln_enc_g[None, :]
    w_x = w_ih[:, H:] * ln_emb_g[None, :]
    b_gates = b_ih + b_hh + w_ih[:, :H] @ ln_enc_b + w_ih[:, H:] @ ln_emb_b

    w_zT = np.ascontiguousarray(
        np.concatenate([w_ctx.T, w_hh.T], axis=0), dtype=np.float32
    )
    x_wT = np.concatenate([w_x.T, b_gates[None, :]], axis=0)
    x_wT = np.ascontiguousarray(x_wT).astype(bf16)

    kwT_b = np.ascontiguousarray(kw_eff.T).astype(bf16)
    qwT_b = np.ascontiguousarray(q_w.T).astype(bf16)
    ewT_b = np.ascontiguousarray(e_w[0][:, None]).astype(bf16)
    qadd_c = np.ascontiguousarray(qadd[:, None], dtype=np.float32)

    in_maps = []
    for c in range(NCORES):
        bsl = slice(BL * c, BL * (c + 1))
        vs = slice(VL * c, VL * (c + 1))
        enc_c = np.ascontiguousarray(enc_hid[bsl].reshape(BS, H), dtype=np.float32)
        ehnT = np.ascontiguousarray(
            enc_hn[:, bsl].transpose(2, 0, 1).reshape(H, NL2 * BL), dtype=np.float32
        )
        ecn = np.ascontiguousarray(
            enc_cn[:, bsl].transpose(1, 0, 2).reshape(BL, NL2 * H), dtype=np.float32
        )
        tgt = np.ascontiguousarray(targets[bsl].T.reshape(TB, 1), dtype=np.int32)
        owT = np.ascontiguousarray(out_w[vs].T).astype(bf16)
        ob = np.ascontiguousarray(out_b[vs][None, :], dtype=np.float32)
        in_maps.append(
            {
                "enc": enc_c,
                "ehnT": ehnT,
                "ecn": ecn,
                "emb": emb,
                "tgt": tgt,
                "kwT": kwT_b,
                "qwT": qwT_b,
                "ewT": ewT_b,
                "qadd": qadd_c,
                "wzT": w_zT,
                "xwT": x_wT,
                "owT": owT,
                "ob": ob,
            }
        )
    return in_maps, consts


_CACHE = {}


def kernel(**inputs) -> np.ndarray:
    in_maps, consts = _prep_inputs(inputs)
    dbgf = bool(int(os.environ.get("KERNEL_DEBUG", "0")))
    key = (dbgf,) + tuple(consts["phw"] + consts["pcw"] + [consts["phb"], consts["pcb"]])
    if key not in _CACHE:
        _CACHE[key] = build_nc(consts, debug=dbgf)
    nc = _CACHE[key]
    res = run_bass_kernel_spmd(
        nc,
        in_maps,
        core_ids=list(range(NCORES)),
        trace=bool(int(os.environ.get("KERNEL_TRACE", "0"))),
    )
    kernel._last = res
    shards = [res.results[c]["out"] for c in range(NCORES)]
    return np.concatenate(shards, axis=2)


kernel._last = None


if __name__ == "__main__":
    shapes = {
        "targets": (B, T),
        "encoder_hidden": (B, S, H),
        "enc_hn": (NL2, B, H),
        "enc_cn": (NL2, B, H),
        "emb": (V, E),
        "ln_enc_g": (H,),
        "ln_enc_b": (H,),
        "ln_emb_g": (E,),
        "ln_emb_b": (E,),
        "proj_hn_w": (1, NL2),
        "proj_hn_b": (1,),
        "proj_cn_w": (1, NL2),
        "proj_cn_b": (1,),
        "q_w": (H, H),
        "q_b": (H,),
        "k_w": (H, H),
        "e_w": (1, H),
        "w_ih": (4 * H, H + E),
        "w_hh": (4 * H, H),
        "b_ih": (4 * H,),
        "b_hh": (4 * H,),
        "out_w": (V, H),
        "out_b": (V,),
    }
    dummy = {
        k: (
            np.zeros(s, np.int64)
            if k == "targets"
            else np.random.RandomState(0).randn(*s).astype(np.float32) * 0.1
        )
        for k, s in shapes.items()
    }
    _, consts = _prep_inputs(dummy)
    nc = build_nc(consts)
    print("build OK")



# revision 23
# speedup vs baseline: 1.1753x; 1.1753x over previous
"""AttentionDecoder Trainium2 kernel — 8-core SPMD, v2.

Strategy:
  - Data-parallel everything: core c owns batch slice [8c, 8c+8) end to end
    (recurrence, attention, and output projection over the FULL vocab).
    No collectives.
  - Attention restructured as matmuls: energies produced in column layout
    [(b,s)-part, 1]; softmax via tanh identity exp(e) = (1+t)/(1-t) with
    t = tanh(e/2) so only one activation-table set (sigmoid_and_others:
    tanh+sigmoid) is ever loaded; the normalized alphas form a
    block-diagonal [BS, BL] matrix A, and the context contribution to the
    LSTM gates is A^T @ M with M = enc_ln^T-projected gate weights
    (M[(b,s), j] = sum_h enc_ln[(b,s), h] * w_ctx[j, h], precomputed once).
  - Gates psum: W_hh*h accumulates early (overlaps attention); ctx part
    (A^T M) lands late; x-part (precomputed per-token gates) added at
    evacuation. Chunk order [g,i,f,o] so cell activations pipeline with
    the matmul chunks.
  - Projection of steps 0-15 is interleaved into steps 16-31 to fill
    tensor-engine stalls; steps 16-31 project in a tail. out_w streamed
    (double-buffered) with the bias as an extra matmul row. Output is
    written bf16 and cast to f32 on the host.
"""

import os
import sys

sys.path.insert(0, "/opt/trn_rl_repo")

import ml_dtypes
import numpy as np

import concourse.bass as bass
from concourse import bacc
import concourse.mybir as mybir
import concourse.tile as tile
from concourse.bass_utils import run_bass_kernel_spmd
from concourse.masks import make_identity

# problem shapes (hardcoded per harness contract)
B, S, H, E, V, NL2, T = 64, 64, 512, 256, 32000, 4, 32
NCORES = 8
BL = B // NCORES  # 8 examples per core
EPS = 1e-5
BS = BL * S  # 512 (b, s) rows per core
TB = T * BL  # 256 (t, b) rows per core
G4 = 4 * H  # 2048 gate dim
KT = H // 128  # 4 partition tiles for hidden dim
VC = 512  # vocab chunk per proj matmul
NVC = V // VC  # 62.5 -> must divide; 32000/512 = 62.5 NO -> use 500
# 32000 = 64 * 500
VCW = 500
NVCW = V // VCW  # 64 vocab chunks
OWG = 4  # vocab chunks per owT stream buffer

F32 = mybir.dt.float32
BF16 = mybir.dt.bfloat16
I32 = mybir.dt.int32
AF = mybir.ActivationFunctionType
ALU = mybir.AluOpType

bf16 = ml_dtypes.bfloat16


def _bc_free(ap, n):
    """Append a step-0 free dim of size n (broadcast along a new inner axis)."""
    return bass.AP(tensor=ap.tensor, offset=ap.offset, ap=[*ap.ap, [0, n]])


def _bc_col(ap, n):
    """[P, 1] column -> [P, n] broadcast (replace free dim with step-0)."""
    return bass.AP(tensor=ap.tensor, offset=ap.offset, ap=[ap.ap[0], [0, n]])


def build_nc(consts, debug=False):
    nc = bacc.Bacc()

    # ---------------- DRAM I/O ----------------
    d_enc = nc.dram_tensor("enc", [BS, H], F32, kind="ExternalInput")
    d_ehnT = nc.dram_tensor("ehnT", [H, NL2 * BL], F32, kind="ExternalInput")
    d_ecn = nc.dram_tensor("ecn", [BL, NL2 * H], F32, kind="ExternalInput")
    d_emb = nc.dram_tensor("emb", [V, E], F32, kind="ExternalInput")
    d_tgt = nc.dram_tensor("tgt", [TB, 1], I32, kind="ExternalInput")
    d_kwT = nc.dram_tensor("kwT", [H, H], BF16, kind="ExternalInput")
    d_qwT = nc.dram_tensor("qwT", [H, H], BF16, kind="ExternalInput")
    d_ewT = nc.dram_tensor("ewT", [H, 1], BF16, kind="ExternalInput")
    d_qadd = nc.dram_tensor("qadd", [H, 1], F32, kind="ExternalInput")
    d_whT = nc.dram_tensor("whT", [H, G4], BF16, kind="ExternalInput")
    d_wcT = nc.dram_tensor("wcT", [H, G4], BF16, kind="ExternalInput")
    d_xwT = nc.dram_tensor("xwT", [2 * 128 + 1, G4], BF16, kind="ExternalInput")
    d_owT = nc.dram_tensor("owT", [H + 1, V], BF16, kind="ExternalInput")
    d_out = nc.dram_tensor("out", [TB, V], BF16, kind="ExternalOutput")

    dbg = {}
    if debug:
        for nm, shp, dt_ in [
            ("dbg_pkT", [H, BS], BF16),
            ("dbg_M", [BS, G4], BF16),
            ("dbg_xg", [TB, G4], BF16),
            ("dbg_esum", [H, BS], BF16),
            ("dbg_expc", [128, KT], F32),
            ("dbg_te", [128, KT], F32),
            ("dbg_en", [128, KT], F32),
            ("dbg_A", [BS, BL], BF16),
            ("dbg_gates", [BL, G4], F32),
            ("dbg_hsT", [H, 8 * (T + 1)], BF16),
        ]:
            dbg[nm] = nc.dram_tensor(nm, shp, dt_, kind="ExternalOutput")

    with tile.TileContext(nc) as tc:
        with (
            tc.tile_pool(name="persist", bufs=1) as P_per,
            tc.tile_pool(name="cell", bufs=2) as P_cell,
            tc.tile_pool(name="outsb", bufs=3) as P_out,
            tc.tile_pool(name="xgd", bufs=2) as P_xg,
            tc.tile_pool(name="ow", bufs=2) as P_ow,
            tc.tile_pool(name="psG", bufs=1, space="PSUM") as PS_g,
            tc.tile_pool(name="psQ", bufs=1, space="PSUM") as PS_q,
            tc.tile_pool(name="psP", bufs=2, space="PSUM") as PS_p,
        ):
            # ---------- persistent SBUF ----------
            id128 = P_per.tile([128, 128], F32, name="id128")
            make_identity(nc, id128[:, :])
            id8b = P_per.tile([8, 8], BF16, name="id8b")
            nc.vector.tensor_copy(id8b[:, :], id128[0:8, 0:8])
            eps_t = P_per.tile([128, 1], F32, name="eps")
            nc.vector.memset(eps_t[:, :], EPS)
            ones_row = P_per.tile([1, 128], BF16, name="ones_row")
            nc.vector.memset(ones_row[:, :], 1.0)
            ones_col = P_per.tile([128, 1], BF16, name="ones_col")
            nc.vector.memset(ones_col[:, :], 1.0)
            # mask01[p, 0] = 1 for p < 64 else 0; col 1 inverted
            mask01 = P_per.tile([128, 2], F32, name="mask01")
            nc.vector.memset(mask01[0:64, 0:1], 1.0)
            nc.vector.memset(mask01[64:128, 0:1], 0.0)
            nc.vector.memset(mask01[0:64, 1:2], 0.0)
            nc.vector.memset(mask01[64:128, 1:2], 1.0)

            # hidden history, transposed: col block t+1 = h_{t+1}; block 0 = h0
            hs_T = [P_per.tile([128, 8 * (T + 1)], BF16, name=f"hsT{k}") for k in range(KT)]
            c_st = P_per.tile([BL, H], F32, name="c_state")
            # block-diag alpha [BS, BL] as 4 partition tiles; zeros persist
            A_t = [P_per.tile([128, BL], BF16, name=f"A{k}") for k in range(KT)]
            for k in range(KT):
                nc.vector.memset(A_t[k][:, :], 0.0)

            # weights resident in SBUF
            qwT = [P_per.tile([128, H], BF16, name=f"qwT{k}") for k in range(KT)]
            ewT = [P_per.tile([128, 1], BF16, name=f"ewT{k}") for k in range(KT)]
            qadd = [P_per.tile([128, 1], F32, name=f"qadd{k}") for k in range(KT)]
            whT = [P_per.tile([128, G4], BF16, name=f"whT{k}") for k in range(KT)]
            for k in range(KT):
                nc.sync.dma_start(qwT[k][:, :], d_qwT[128 * k : 128 * (k + 1), :])
                nc.sync.dma_start(ewT[k][:, :], d_ewT[128 * k : 128 * (k + 1), :])
                nc.sync.dma_start(qadd[k][:, :], d_qadd[128 * k : 128 * (k + 1), :])
                nc.sync.dma_start(whT[k][:, :], d_whT[128 * k : 128 * (k + 1), :])

            pk_Tb = [P_per.tile([128, BS], BF16, name=f"pkT{g}") for g in range(KT)]
            esum = [P_per.tile([128, BS], BF16, name=f"esum{g}") for g in range(KT)]
            M_t = [P_per.tile([128, G4], BF16, name=f"M{m}") for m in range(KT)]
            x_gates = [P_per.tile([128, G4], BF16, name=f"xg{m}") for m in range(2)]

            # ========== precompute (scoped pools) ==========
            with (
                tc.tile_pool(name="pre", bufs=1) as P_pre,
                tc.tile_pool(name="prew", bufs=2) as P_pw,
            ):
                kwT = [P_pre.tile([128, H], BF16, name=f"kwT{k}") for k in range(KT)]
                wcT = [P_pre.tile([128, G4], BF16, name=f"wcT{k}") for k in range(KT)]
                for k in range(KT):
                    nc.sync.dma_start(kwT[k][:, :], d_kwT[128 * k : 128 * (k + 1), :])
                    nc.sync.dma_start(wcT[k][:, :], d_wcT[128 * k : 128 * (k + 1), :])
                xwT = [P_pre.tile([128, G4], BF16, name=f"xwT{k}") for k in range(2)]
                xwTb = P_pre.tile([1, G4], BF16, name="xwTbias")
                for k in range(2):
                    nc.sync.dma_start(xwT[k][:, :], d_xwT[128 * k : 128 * (k + 1), :])
                nc.sync.dma_start(xwTb[:, :], d_xwT[2 * 128 : 2 * 128 + 1, :])

                # ----- h0 / c0 -----
                ehnT = [
                    P_pre.tile([128, NL2 * BL], F32, name=f"ehnT{k}") for k in range(KT)
                ]
                for k in range(KT):
                    nc.sync.dma_start(ehnT[k][:, :], d_ehnT[128 * k : 128 * (k + 1), :])
                ecn = P_pre.tile([BL, NL2 * H], F32, name="ecn")
                nc.sync.dma_start(ecn[:, :], d_ecn[:, :])

                phw, pcw = consts["phw"], consts["pcw"]
                for k in range(KT):
                    h0f = P_pw.tile([128, BL], F32, name="h0f")
                    tmp = P_pw.tile([128, BL], F32, name="h0tmp")
                    nc.vector.tensor_scalar_mul(
                        h0f[:, :], ehnT[k][:, 0:BL], float(phw[0])
                    )
                    for l in range(1, NL2):
                        nc.vector.tensor_scalar_mul(
                            tmp[:, :], ehnT[k][:, BL * l : BL * (l + 1)], float(phw[l])
                        )
                        nc.vector.tensor_add(h0f[:, :], h0f[:, :], tmp[:, :])
                    nc.vector.tensor_scalar(
                        out=hs_T[k][:, 0:BL],
                        in0=h0f[:, :],
                        scalar1=float(consts["phb"]),
                        scalar2=None,
                        op0=ALU.add,
                    )

                ctmp = P_pw.tile([BL, H], F32, name="c0tmp")
                nc.vector.tensor_scalar_mul(c_st[:, :], ecn[:, 0:H], float(pcw[0]))
                for l in range(1, NL2):
                    nc.vector.tensor_scalar_mul(
                        ctmp[:, :], ecn[:, H * l : H * (l + 1)], float(pcw[l])
                    )
                    nc.vector.tensor_add(c_st[:, :], c_st[:, :], ctmp[:, :])
                nc.vector.tensor_scalar_add(
                    c_st[:, :], c_st[:, :], float(consts["pcb"])
                )

                # ----- encoder LN (natural layout) + transpose -----
                enc_ln = [P_pre.tile([128, H], F32, name=f"encln{i}") for i in range(4)]
                for i in range(BS // 128):
                    x_t = P_pw.tile([128, H], F32, name="enc_in")
                    nc.sync.dma_start(x_t[:, :], d_enc[128 * i : 128 * (i + 1), :])
                    stats = P_pw.tile([128, 6], F32, name="enc_st")
                    mv = P_pw.tile([128, 2], F32, name="enc_mv")
                    nc.vector.bn_stats(out=stats[:, :], in_=x_t[:, :])
                    nc.vector.bn_aggr(out=mv[:, :], in_=stats[:, :])
                    nc.scalar.activation(
                        out=mv[:, 1:2], in_=mv[:, 1:2], func=AF.Sqrt, bias=eps_t[:, :]
                    )
                    nc.vector.reciprocal(out=mv[:, 1:2], in_=mv[:, 1:2])
                    nc.vector.tensor_scalar(
                        out=enc_ln[i][:, :],
                        in0=x_t[:, :],
                        scalar1=mv[:, 0:1],
                        scalar2=mv[:, 1:2],
                        op0=ALU.subtract,
                        op1=ALU.mult,
                    )
                enc_Tb = [P_pre.tile([128, BS], BF16, name=f"encT{k}") for k in range(KT)]
                for i in range(4):
                    for j in range(4):
                        pt = PS_p.tile([128, 512], F32, name="big_ps")
                        nc.tensor.transpose(
                            out=pt[:, 0:128],
                            in_=enc_ln[i][:, 128 * j : 128 * (j + 1)],
                            identity=id128[:, :],
                        )
                        nc.vector.tensor_copy(
                            enc_Tb[j][:, 128 * i : 128 * (i + 1)], pt[:, 0:128]
                        )

                # ----- projected keys pk_T (bf16) -----
                for g in range(KT):
                    pp = PS_p.tile([128, 512], F32, name="big_ps")
                    for k in range(KT):
                        nc.tensor.matmul(
                            pp[:, :],
                            lhsT=kwT[k][:, 128 * g : 128 * (g + 1)],
                            rhs=enc_Tb[k][:, :],
                            start=(k == 0),
                            stop=(k == KT - 1),
                        )
                    nc.vector.tensor_copy(pk_Tb[g][:, :], pp[:, :])

                # ----- M = per-(b,s)-row ctx gate weights -----
                # M[(bs)-tile m][p, j] = sum_h enc_ln[(bs), h] * wc[j, h]
                for m in range(KT):
                    for c in range(4):
                        pp = PS_p.tile([128, 512], F32, name="big_ps")
                        for k in range(KT):
                            nc.tensor.matmul(
                                pp[:, :],
                                lhsT=enc_Tb[k][:, 128 * m : 128 * (m + 1)],
                                rhs=wcT[k][:, 512 * c : 512 * (c + 1)],
                                start=(k == 0),
                                stop=(k == KT - 1),
                            )
                        nc.vector.tensor_copy(M_t[m][:, 512 * c : 512 * (c + 1)], pp[:, :])

                # ----- embedding gather + LN + transpose + x_gates -----
                xe_ln = [P_pre.tile([128, E], F32, name=f"xeln{i}") for i in range(2)]
                for i in range(2):
                    tgt_sb = P_pw.tile([128, 1], I32, name="tgt")
                    nc.sync.dma_start(tgt_sb[:, :], d_tgt[128 * i : 128 * (i + 1), :])
                    xg = P_pw.tile([128, E], F32, name="xemb")
                    nc.gpsimd.indirect_dma_start(
                        out=xg[:, :],
                        out_offset=None,
                        in_=d_emb[:, :],
                        in_offset=bass.IndirectOffsetOnAxis(ap=tgt_sb[:, 0:1], axis=0),
                    )
                    stats = P_pw.tile([128, 6], F32, name="xe_st")
                    mv = P_pw.tile([128, 2], F32, name="xe_mv")
                    nc.vector.bn_stats(out=stats[:, :], in_=xg[:, :])
                    nc.vector.bn_aggr(out=mv[:, :], in_=stats[:, :])
                    nc.scalar.activation(
                        out=mv[:, 1:2], in_=mv[:, 1:2], func=AF.Sqrt, bias=eps_t[:, :]
                    )
                    nc.vector.reciprocal(out=mv[:, 1:2], in_=mv[:, 1:2])
                    nc.vector.tensor_scalar(
                        out=xe_ln[i][:, :],
                        in0=xg[:, :],
                        scalar1=mv[:, 0:1],
                        scalar2=mv[:, 1:2],
                        op0=ALU.subtract,
                        op1=ALU.mult,
                    )
                xlnT = [P_pre.tile([128, TB], BF16, name=f"xlnT{k}") for k in range(2)]
                for i in range(2):
                    for j in range(2):
                        pt = PS_p.tile([128, 512], F32, name="big_ps")
                        nc.tensor.transpose(
                            out=pt[:, 0:128],
                            in_=xe_ln[i][:, 128 * j : 128 * (j + 1)],
                            identity=id128[:, :],
                        )
                        nc.vector.tensor_copy(
                            xlnT[j][:, 128 * i : 128 * (i + 1)], pt[:, 0:128]
                        )

                for m in range(2):
                    for c in range(4):
                        pp = PS_p.tile([128, 512], F32, name="big_ps")
                        for k in range(2):
                            nc.tensor.matmul(
                                pp[:, :],
                                lhsT=xlnT[k][:, 128 * m : 128 * (m + 1)],
                                rhs=xwT[k][:, 512 * c : 512 * (c + 1)],
                                start=(k == 0),
                                stop=False,
                            )
                        nc.tensor.matmul(
                            pp[:, :],
                            lhsT=ones_row[:, :],
                            rhs=xwTb[:, 512 * c : 512 * (c + 1)],
                            start=False,
                            stop=True,
                        )
                        nc.vector.tensor_copy(
                            x_gates[m][:, 512 * c : 512 * (c + 1)], pp[:, :]
                        )

            if debug:
                for k in range(KT):
                    nc.sync.dma_start(dbg["dbg_pkT"][128 * k : 128 * (k + 1), :], pk_Tb[k][:, :])
                    nc.sync.dma_start(dbg["dbg_M"][128 * k : 128 * (k + 1), :], M_t[k][:, :])
                for m in range(2):
                    nc.sync.dma_start(dbg["dbg_xg"][128 * m : 128 * (m + 1), :], x_gates[m][:, :])

            # owT stream state: buffers of OWG vocab chunks
            ow_bufs = {}

            def emit_ow_dma(grp):
                """DMA owT columns for vocab-chunk group grp into a rotating buffer."""
                cw = VCW * OWG
                col0 = cw * grp
                tiles = [P_ow.tile([128, cw], BF16, name=f"ow{k}") for k in range(KT)]
                obr = P_ow.tile([1, cw], BF16, name="ow_b")
                for k in range(KT):
                    nc.sync.dma_start(
                        tiles[k][:, :], d_owT[128 * k : 128 * (k + 1), col0 : col0 + cw]
                    )
                nc.sync.dma_start(obr[:, :], d_owT[H : H + 1, col0 : col0 + cw])
                ow_bufs[grp] = (tiles, obr)

            def emit_proj(vc, mt):
                """Project vocab chunk vc for row block mt (0: steps 0-15, 1: 16-31)."""
                tiles, obr = ow_bufs[vc // OWG]
                coff = VCW * (vc % OWG)
                pp = PS_p.tile([128, 512], F32, name="big_ps")
                for k in range(KT):
                    nc.tensor.matmul(
                        pp[:, 0:VCW],
                        lhsT=hs_T[k][:, 8 + 128 * mt : 8 + 128 * (mt + 1)],
                        rhs=tiles[k][:, coff : coff + VCW],
                        start=(k == 0),
                        stop=False,
                    )
                nc.tensor.matmul(
                    pp[:, 0:VCW],
                    lhsT=ones_row[:, :],
                    rhs=obr[:, coff : coff + VCW],
                    start=False,
                    stop=True,
                )
                ob = P_out.tile([128, VCW], BF16, name="proj_out")
                nc.vector.tensor_copy(ob[:, :], pp[:, 0:VCW])
                nc.sync.dma_start(
                    d_out[128 * mt : 128 * (mt + 1), VCW * vc : VCW * (vc + 1)],
                    ob[:, :],
                )

            # ================= recurrence =================
            # gates psum: 4 chunks [g, i, f, o], chunk c on partitions 8c..8c+8
            ps_g = [PS_g.tile([BL, 512], F32, name=f"g_ps{c}") for c in range(4)]
            for t in range(T):
                h_blk = [hs_T[k][:, 8 * t : 8 * (t + 1)] for k in range(KT)]

                # psum scratch: q 0:32, e partials 32:48, r_bc 48:56, sums 56:64
                ps_m = PS_q.tile([128, 64], F32, name="misc_ps")

                # prefetch this step's x-gates rows (engine ops cannot read
                # SBUF at partition offsets that aren't multiples of 32)
                xg_t = P_xg.tile([BL, G4], BF16, name="xg_t")
                nc.sync.dma_start(
                    xg_t[:, :],
                    x_gates[t // 16][BL * (t % 16) : BL * (t % 16 + 1), :],
                )

                # --- q = qw^T h (column layout) ---
                for g in range(KT):
                    for k in range(KT):
                        nc.tensor.matmul(
                            ps_m[:, 8 * g : 8 * (g + 1)],
                            lhsT=qwT[k][:, 128 * g : 128 * (g + 1)],
                            rhs=h_blk[k],
                            start=(k == 0),
                            stop=(k == KT - 1),
                        )

                # --- h-part of gates (early; overlaps attention) ---
                for c in range(4):
                    for k in range(KT):
                        nc.tensor.matmul(
                            ps_g[c][:, :],
                            lhsT=h_blk[k],
                            rhs=whT[k][:, 512 * c : 512 * (c + 1)],
                            start=(k == 0),
                            stop=False,
                            skip_group_check=True,
                        )

                # --- q evac (+bias) ---
                q_sb = P_cell.tile([128, 32], BF16, name="q_sb")
                for g in range(KT):
                    nc.vector.tensor_scalar(
                        out=q_sb[:, 8 * g : 8 * (g + 1)],
                        in0=ps_m[:, 8 * g : 8 * (g + 1)],
                        scalar1=qadd[g][:, 0:1],
                        scalar2=None,
                        op0=ALU.add,
                    )

                # --- esum = pk + q (broadcast over s); tanh in place ---
                for g in range(KT):
                    eng = nc.vector if g % 2 == 0 else nc.gpsimd
                    eng.tensor_tensor(
                        out=esum[g][:, :].rearrange("p (b s) -> p b s", s=S),
                        in0=pk_Tb[g][:, :].rearrange("p (b s) -> p b s", s=S),
                        in1=_bc_free(q_sb[:, 8 * g : 8 * (g + 1)], S),
                        op=ALU.add,
                    )
                    nc.scalar.activation(
                        out=esum[g][:, :], in_=esum[g][:, :], func=AF.Tanh
                    )
                    # energy partials: col 32+4*mt+g (one accumulation group
                    # per psum column -- same-bank groups must not interleave)
                    for mt in range(KT):
                        nc.tensor.matmul(
                            ps_m[:, 32 + 4 * mt + g : 33 + 4 * mt + g],
                            lhsT=esum[g][:, 128 * mt : 128 * (mt + 1)],
                            rhs=ewT[g][:, :],
                            start=True,
                            stop=True,
                        )

                # --- reduce g-partials, then exp via tanh identity ---
                en4 = P_cell.tile([128, KT], F32, name="en4")
                nc.vector.tensor_reduce(
                    out=en4[:, :],
                    in_=ps_m[:, 32:48].rearrange("p (m g) -> p m g", g=KT),
                    axis=mybir.AxisListType.X,
                    op=ALU.add,
                )
                t_e = P_cell.tile([128, KT], F32, name="t_e")
                nc.scalar.activation(
                    out=t_e[:, :], in_=en4[:, :], func=AF.Tanh, scale=0.5
                )
                u_e = P_cell.tile([128, KT], F32, name="u_e")
                v_e = P_cell.tile([128, KT], F32, name="v_e")
                expc = P_cell.tile([128, KT], F32, name="expc")
                nc.vector.tensor_scalar_add(u_e[:, :], t_e[:, :], 1.0)
                nc.gpsimd.tensor_scalar(
                    out=v_e[:, :],
                    in0=t_e[:, :],
                    scalar1=-1.0,
                    scalar2=1.0,
                    op0=ALU.mult,
                    op1=ALU.add,
                )
                nc.vector.reciprocal(out=v_e[:, :], in_=v_e[:, :])
                nc.vector.tensor_mul(expc[:, :], u_e[:, :], v_e[:, :])

                # --- block-diag alpha (unnormalized): A[kt][:, 2kt:2kt+2] ---
                for k in range(KT):
                    eng = nc.vector if k % 2 == 0 else nc.gpsimd
                    eng.tensor_tensor(
                        out=A_t[k][:, 2 * k : 2 * k + 2],
                        in0=_bc_col(expc[:, k : k + 1], 2),
                        in1=mask01[:, :],
                        op=ALU.mult,
                    )
                # sums over s per example -> [1, BL]
                for k in range(KT):
                    nc.tensor.matmul(
                        ps_m[0:1, 56:64],
                        lhsT=ones_col[:, :],
                        rhs=A_t[k][:, :],
                        start=(k == 0),
                        stop=(k == KT - 1),
                    )
                r_row = P_cell.tile([1, BL], F32, name="r_row")
                nc.vector.reciprocal(out=r_row[:, :], in_=ps_m[0:1, 56:64])
                r_b16 = P_cell.tile([1, BL], BF16, name="r_b16")
                nc.gpsimd.tensor_copy(r_b16[:, :], r_row[:, :])
                # broadcast recip sums down partitions via outer product
                nc.tensor.matmul(
                    ps_m[:, 48:56],
                    lhsT=ones_row[:, :],
                    rhs=r_b16[:, :],
                    start=True,
                    stop=True,
                )
                # normalize A in place (off-block zeros stay zero)
                for k in range(KT):
                    nc.vector.tensor_tensor(
                        out=A_t[k][:, :],
                        in0=A_t[k][:, :],
                        in1=ps_m[:, 48:56],
                        op=ALU.mult,
                    )
                if debug and t == 0:
                    nc.sync.dma_start(dbg["dbg_en"][:, :], en4[:, :])
                    nc.sync.dma_start(dbg["dbg_te"][:, :], t_e[:, :])
                    nc.sync.dma_start(dbg["dbg_expc"][:, :], expc[:, :])
                    for k in range(KT):
                        nc.sync.dma_start(dbg["dbg_A"][128 * k : 128 * (k + 1), :], A_t[k][:, :])
                    for k in range(KT):
                        nc.sync.dma_start(dbg["dbg_esum"][128 * k : 128 * (k + 1), :], esum[k][:, :])

                # --- ctx part of gates: psum += A^T M ---
                for c in range(4):
                    for k in range(KT):
                        nc.tensor.matmul(
                            ps_g[c][:, :],
                            lhsT=A_t[k][:, :],
                            rhs=M_t[k][:, 512 * c : 512 * (c + 1)],
                            start=False,
                            stop=(k == KT - 1),
                            skip_group_check=True,
                        )

                # --- gates evac (+x part) and cell, chunk order [g, i, f, o] ---
                gates = P_cell.tile([BL, G4], F32, name="gates")
                for c in range(4):
                    nc.vector.tensor_tensor(
                        out=gates[:, 512 * c : 512 * (c + 1)],
                        in0=ps_g[c][:, :],
                        in1=xg_t[:, 512 * c : 512 * (c + 1)],
                        op=ALU.add,
                    )
                    nc.scalar.activation(
                        out=gates[:, 512 * c : 512 * (c + 1)],
                        in_=gates[:, 512 * c : 512 * (c + 1)],
                        func=AF.Tanh if c == 0 else AF.Sigmoid,
                    )
                tg, si = gates[:, 0:512], gates[:, 512:1024]
                sf, so = gates[:, 1024:1536], gates[:, 1536:2048]
                m2 = P_cell.tile([BL, H], F32, name="m2")
                m1 = P_cell.tile([BL, H], F32, name="m1")
                nc.vector.tensor_mul(m2[:, :], si, tg)
                nc.gpsimd.tensor_mul(m1[:, :], sf, c_st[:, :])
                nc.vector.tensor_add(c_st[:, :], m1[:, :], m2[:, :])
                tc2 = P_cell.tile([BL, H], F32, name="tc2")
                nc.scalar.activation(out=tc2[:, :], in_=c_st[:, :], func=AF.Tanh)
                h2b = P_cell.tile([BL, H], BF16, name="h2b")
                nc.gpsimd.tensor_mul(h2b[:, :], so, tc2[:, :])

                if debug and t == 0:
                    nc.sync.dma_start(dbg["dbg_gates"][:, :], gates[:, :])

                # --- projection interleave: fill PE stall before transposes ---
                if t == 15:
                    emit_ow_dma(0)  # prefetch first owT buffer
                elif t >= 16:
                    grp = t - 16
                    if t < 31:
                        emit_ow_dma(grp + 1)  # prefetch next buffer
                    for vc in range(OWG * grp, OWG * grp + OWG):
                        emit_proj(vc, 0)

                # --- h2 transpose into history ---
                for k in range(KT):
                    pt = PS_q.tile([128, BL], BF16, name="h_tp")
                    nc.tensor.transpose(
                        out=pt[:, :],
                        in_=h2b[:, 128 * k : 128 * (k + 1)],
                        identity=id8b[:, :],
                    )
                    nc.vector.tensor_copy(hs_T[k][:, 8 * (t + 1) : 8 * (t + 2)], pt[:, :])

            if debug:
                for k in range(KT):
                    nc.sync.dma_start(dbg["dbg_hsT"][128 * k : 128 * (k + 1), :], hs_T[k][:, :])

            # ================= projection tail: steps 16-31 =================
            for grp in range(NVCW // OWG):
                emit_ow_dma(grp)  # re-stream this group's owT columns
                for vc in range(OWG * grp, OWG * grp + OWG):
                    emit_proj(vc, 1)

    nc.compile()
    return nc


def _prep_inputs(inputs):
    """Host-side layout prep. Returns (in_maps, consts)."""
    f = lambda x: np.asarray(x, dtype=np.float32)
    targets = np.asarray(inputs["targets"])
    enc_hid = f(inputs["encoder_hidden"])
    enc_hn = f(inputs["enc_hn"])
    enc_cn = f(inputs["enc_cn"])
    emb = f(inputs["emb"])
    ln_enc_g = f(inputs["ln_enc_g"])
    ln_enc_b = f(inputs["ln_enc_b"])
    ln_emb_g = f(inputs["ln_emb_g"])
    ln_emb_b = f(inputs["ln_emb_b"])
    q_w = f(inputs["q_w"])
    q_b = f(inputs["q_b"])
    k_w = f(inputs["k_w"])
    e_w = f(inputs["e_w"])
    w_ih = f(inputs["w_ih"])
    w_hh = f(inputs["w_hh"])
    b_ih = f(inputs["b_ih"])
    b_hh = f(inputs["b_hh"])
    out_w = f(inputs["out_w"])
    out_b = f(inputs["out_b"])

    consts = dict(
        phw=[float(x) for x in f(inputs["proj_hn_w"])[0]],
        phb=float(f(inputs["proj_hn_b"])[0]),
        pcw=[float(x) for x in f(inputs["proj_cn_w"])[0]],
        pcb=float(f(inputs["proj_cn_b"])[0]),
    )

    # fold LN affines into adjacent matmuls
    kw_eff = k_w * ln_enc_g[None, :]
    qadd = q_b + k_w @ ln_enc_b
    w_ctx = w_ih[:, :H] * ln_enc_g[None, :]
    w_x = w_ih[:, H:] * ln_emb_g[None, :]
    b_gates = b_ih + b_hh + w_ih[:, :H] @ ln_enc_b + w_ih[:, H:] @ ln_emb_b

    # reorder gate rows [i, f, g, o] -> [g, i, f, o]
    perm = np.concatenate(
        [np.arange(2 * H, 3 * H), np.arange(0, H), np.arange(H, 2 * H),
         np.arange(3 * H, 4 * H)]
    )
    w_ctx = w_ctx[perm]
    w_hh_r = w_hh[perm]
    w_x = w_x[perm]
    b_gates = b_gates[perm]

    whT_b = np.ascontiguousarray(w_hh_r.T).astype(bf16)
    wcT_b = np.ascontiguousarray(w_ctx.T).astype(bf16)
    x_wT = np.concatenate([w_x.T, b_gates[None, :]], axis=0)
    x_wT = np.ascontiguousarray(x_wT).astype(bf16)

    kwT_b = np.ascontiguousarray(kw_eff.T).astype(bf16)
    qwT_b = np.ascontiguousarray(q_w.T).astype(bf16)
    ewT_b = np.ascontiguousarray(e_w[0][:, None]).astype(bf16)
    qadd_c = np.ascontiguousarray(qadd[:, None], dtype=np.float32)
    owT_b = np.ascontiguousarray(
        np.concatenate([out_w.T, out_b[None, :]], axis=0)
    ).astype(bf16)

    in_maps = []
    for c in range(NCORES):
        bsl = slice(BL * c, BL * (c + 1))
        enc_c = np.ascontiguousarray(enc_hid[bsl].reshape(BS, H), dtype=np.float32)
        ehnT = np.ascontiguousarray(
            enc_hn[:, bsl].transpose(2, 0, 1).reshape(H, NL2 * BL), dtype=np.float32
        )
        ecn = np.ascontiguousarray(
            enc_cn[:, bsl].transpose(1, 0, 2).reshape(BL, NL2 * H), dtype=np.float32
        )
        tgt = np.ascontiguousarray(targets[bsl].T.reshape(TB, 1), dtype=np.int32)
        in_maps.append(
            {
                "enc": enc_c,
                "ehnT": ehnT,
                "ecn": ecn,
                "emb": emb,
                "tgt": tgt,
                "kwT": kwT_b,
                "qwT": qwT_b,
                "ewT": ewT_b,
                "qadd": qadd_c,
                "whT": whT_b,
                "wcT": wcT_b,
                "xwT": x_wT,
                "owT": owT_b,
            }
        )
    return in_maps, consts


_CACHE = {}


def kernel(**inputs) -> np.ndarray:
    in_maps, consts = _prep_inputs(inputs)
    dbgf = bool(int(os.environ.get("KERNEL_DEBUG", "0")))
    key = (dbgf,) + tuple(consts["phw"] + consts["pcw"] + [consts["phb"], consts["pcb"]])
    if key not in _CACHE:
        _CACHE[key] = build_nc(consts, debug=dbgf)
    nc = _CACHE[key]
    res = run_bass_kernel_spmd(
        nc,
        in_maps,
        core_ids=list(range(NCORES)),
        trace=bool(int(os.environ.get("KERNEL_TRACE", "0"))),
    )
    kernel._last = res
    # per-core out: [TB=(t b), V] bf16 -> [BL, T, V] f32
    shards = []
    for c in range(NCORES):
        o = np.asarray(res.results[c]["out"], dtype=np.float32)
        shards.append(o.reshape(T, BL, V).transpose(1, 0, 2))
    return np.concatenate(shards, axis=0)


kernel._last = None


if __name__ == "__main__":
    shapes = {
        "targets": (B, T),
        "encoder_hidden": (B, S, H),
        "enc_hn": (NL2, B, H),
        "enc_cn": (NL2, B, H),
        "emb": (V, E),
        "ln_enc_g": (H,),
        "ln_enc_b": (H,),
        "ln_emb_g": (E,),
        "ln_emb_b": (E,),
        "proj_hn_w": (1, NL2),
        "proj_hn_b": (1,),
        "proj_cn_w": (1, NL2),
        "proj_cn_b": (1,),
        "q_w": (H, H),
        "q_b": (H,),
        "k_w": (H, H),
        "e_w": (1, H),
        "w_ih": (4 * H, H + E),
        "w_hh": (4 * H, H),
        "b_ih": (4 * H,),
        "b_hh": (4 * H,),
        "out_w": (V, H),
        "out_b": (V,),
    }
    dummy = {
        k: (
            np.zeros(s, np.int64)
            if k == "targets"
            else np.random.RandomState(0).randn(*s).astype(np.float32) * 0.1
        )
        for k, s in shapes.items()
    }
    _, consts = _prep_inputs(dummy)
    nc = build_nc(consts)
    print("build OK")
